# revision 25
# baseline (speedup 1.0000x reference)
"""Correlation cost-volume kernel for Trainium2 (8 NeuronCores).

out[b,d,h,w] = sum_c left[b,c,h,w] * right[b,c,h,w-shift[d]]
  left/right: [4, 64, 256, 512] f32, shift: arange(96) -> out [4, 96, 256, 512] f32

Strategy:
  - Shard (b, h-half) across 8 cores: per-core left/right [64, 128, 512], no halo
    (shifts are along W only), no collectives.
  - Per (h, w-chunk of 128): two TensorEngine matmuls [K=64, M=64, N=159] bf16
    compute the Gram band G[i, c] = sum_ch L[ch, w0+i] * R[ch, r0+c] for the
    two 64-wide sub-chunks, with the upper sub-chunk's rhs window shifted by
    64 and placed at tile_position col 64, so both halves land band-ALIGNED in
    one PSUM region [128, 159] (partition i needs cols [i%64, i%64+96)).
    M=64 matters twice: the stream runs at full PE clock (M=128 streams at
    half rate - PSUM write port is 64 f32/cycle), and band alignment makes the
    extraction full-partition-width. Two h rows (even/odd) run on K-partition
    halves via tile_position rows 0/64. t=0 windows that would read w<0 are
    trimmed (no zero pad in SBUF); the host zeroes the w<d triangle instead.
  - Band extraction: ONE copy per (h-pair, parity) moves all 4 chunks
    [128, 4x159] PSUM -> SBUF bf16 (4D AP over 2 banks x 2 chunks), alternating
    Vector/Scalar engines (GpSimd cannot access PSUM).
  - Output ships as the 159-wide skewed rectangles (20.8MB/core, one DMA per
    h-pair with 2544B runs); the de-skew (banded gather) happens on the host
    with one as_strided view - no DRAM scratch roundtrip on device.
  - Host: pack/cast inputs to bf16, gather + upcast + transpose the output.
  Measured ~117us on HW (baseline 260611ns quoted / ~208754ns measured):
  PE-floor-bound (1024 matmul slots x ~114ns: 54ns stream + ~60ns LDWEIGHTS
  that this walrus build cannot hide; --enable-ldw-opt crashes its codegen),
  with DMA (37.6MB at ~360GB/s burst) just underneath.
"""
import sys

sys.path.insert(0, "/opt/trn_rl_repo")

import numpy as np
import ml_dtypes

import concourse.bass as bass
import concourse.bass_utils as _bass_utils
import concourse.mybir as mybir
import concourse.tile as tile
from concourse.ap import AP
from concourse.bass_utils import run_bass_kernel_spmd
from concourse.vector_clock import ScopedClock

# note: --enable-ldw-opt=true (walrus LDWEIGHTS ping-pong) crashes this
# walrus build's codegen; keep the default false.

B, C, H, W, D = 4, 64, 256, 512, 96
HC = H // 2          # 128 h rows per core
NP = HC // 2         # 64 h-pairs per core
NT = 4               # w-chunks of 128 per h row
NG = 128 + D - 1     # 223 gram cols per chunk
NB = 64 + D - 1      # 159 band-window cols per 64-row half
PAIR_COLS = W + W    # 512 R + 512 L = 1024 (no zero pad: t=0 windows are
                     # trimmed to valid cols; host zeroes the w<d triangle)
L_OFF = W            # L data starts at col 512; R at col 0
BLKP = 8             # h-pairs per input block
NBLK = NP // BLKP    # 8 blocks
ROW = 2 * NT * NB    # out_sb cols per h-pair: (par, t, c) = 2*4*159 = 1272
PS_COLS = 1024       # psum tile: 2 banks, chunks at cols 0/159/512/671

BF16 = mybir.dt.bfloat16
F32 = mybir.dt.float32


_orig_add_instruction = tile.TileContext._add_instruction


def _patched_add_instruction(self, inst):
    # This walrus build allows at most ONE sync-wait per instruction: peel
    # extra waits onto single-wait NOPs on the same engine, just before it.
    si = inst.sync_info
    if si is not None and len(si.on_wait) > 1:
        waits = list(si.on_wait)
        for w in waits[:-1]:
            nop = mybir.InstNoOp(
                name=self.nc.get_next_instruction_name(),
                text_hint="split_wait",
                bass_nofuse=True,
            )
            nop.engine = inst.engine
            nop.sync_info = mybir.SyncInfo(on_wait=[w], on_update=[])
            _orig_add_instruction(self, nop)
        si.on_wait = waits[-1:]
    _orig_add_instruction(self, inst)


tile.TileContext._add_instruction = _patched_add_instruction


def _patched_drain_and_barrier(self, tick_clock, wait_clock):
    # This walrus build allows only ONE sync-wait on the tail Drain CTRL
    # instruction; split the final-clock waits across single-wait NOPs.
    nc = self.nc
    probe = nc.sync.nop(nofuse=True, hint="drain_waits")
    wait_clock.add_sem_waits(probe.ins, ScopedClock({None: tick_clock.global_clock}))
    waits = list(probe.ins.sync_info.on_wait)
    probe.ins.sync_info.on_wait = waits[:1]
    for w in waits[1:]:
        n = nc.sync.nop(nofuse=True, hint="drain_waits")
        n.ins.sync_info = mybir.SyncInfo(on_wait=[w], on_update=[])
    nc.sync.drain()
    nc.all_engine_barrier()
    assert self.sems is not None
    popped = nc._tile_sem_poison_stack.pop()
    assert popped is self._sem_poison
    nc.clear_and_free_semaphores(list(self.sems.allocated().values()))
    nc.all_engine_barrier()


tile.TileContext._drain_and_barrier = _patched_drain_and_barrier


# chunk t lives at psum col CH_OFF[t] of a [128, PS_COLS] tile (2 banks,
# 2 chunks per bank: matmul output must not straddle a 512-f32 bank boundary)
CH_OFF = [0, NB, 512, 512 + NB]


def build_graph():
    nc = bass.Bass()
    lr_ext = nc.declare_dram_parameter("lrpack", [128, NP, 2 * W], BF16, isOutput=False)
    out_ext = nc.declare_dram_parameter("out", [128, NP * ROW], BF16, isOutput=True)

    with tile.TileContext(nc) as tc:
        with (
            tc.tile_pool(name="inp", bufs=3) as in_pool,
            tc.tile_pool(name="outsb", bufs=6) as out_pool,
            tc.tile_pool(name="psum", bufs=4, space="PSUM") as psum_pool,
        ):
            # GPSIMD/Pool cannot access PSUM (verifier); DVE + Act only
            copy_fns = [
                lambda d, s: nc.vector.tensor_copy(d, s),
                lambda d, s: nc.scalar.copy(d, s),
            ]
            ce = 0  # round-robin cursor
            for blk in range(NBLK):
                # ---- load one block: 8 h-pairs -------------------------------
                blk_tile = in_pool.tile([128, BLKP * PAIR_COLS], BF16)
                h2_0 = blk * BLKP
                # host packs R||L contiguously: one DMA, 2048-byte runs
                # (issuing on the Act HWDGE queue instead was measured slower:
                # the DMA descriptor generation contends with the band copies)
                src_rl = lr_ext[:, h2_0 : h2_0 + BLKP, :]
                nc.sync.dma_start(blk_tile[:], src_rl)

                # ---- compute: per h-pair, 2 parities x 4 w-chunks ------------
                for j2 in range(BLKP):
                    base = j2 * PAIR_COLS
                    out_sb = out_pool.tile([128, ROW], BF16)
                    for par in range(2):
                        p0 = 64 * par
                        ps = psum_pool.tile([128, PS_COLS], F32)
                        for t in range(NT):
                            w0 = 128 * t
                            for half in range(2):
                                # M=64 sub-chunk, band-aligned: rhs window is
                                # shifted by 64 for the upper half so both
                                # halves land in psum cols [CH_OFF[t], +159).
                                # t=0 windows start before R col 0: trim to the
                                # valid cols (c0 offset); the skipped psum cols
                                # are the w<d region, zeroed on the host.
                                lw = base + L_OFF + w0 + 64 * half
                                lhsT = blk_tile[p0 : p0 + 64, lw : lw + 64]
                                r0 = w0 + 64 * half - (D - 1)
                                c0 = max(0, -r0)
                                rhs = blk_tile[
                                    p0 : p0 + 64,
                                    base + r0 + c0 : base + r0 + NB,
                                ]
                                nc.tensor.matmul(
                                    ps[
                                        64 * half : 64 * half + 64,
                                        CH_OFF[t] + c0 : CH_OFF[t] + NB,
                                    ],
                                    lhsT=lhsT,
                                    rhs=rhs,
                                    start=True,
                                    stop=True,
                                    tile_position=(p0, 64 * half),
                                )
                        # one full-partition band copy per (pair, par): the 4
                        # chunks via 4D AP [(part)(bank)(chunk-in-bank)(col)]
                        pw = ps.tensor.shape[1]
                        ow = out_sb.tensor.shape[1]
                        src = AP(
                            tensor=ps.tensor,
                            offset=ps.offset,
                            ap=[[pw, 128], [512, 2], [NB, 2], [1, NB]],
                        )
                        dst = AP(
                            tensor=out_sb.tensor,
                            offset=out_sb.offset + par * NT * NB,
                            ap=[[ow, 128], [2 * NB, 2], [NB, 2], [1, NB]],
                        )
                        copy_fns[ce % 2](dst, src)
                        ce += 1
                    # one DMA per pair: contiguous 2544-B runs per partition
                    gp = blk * BLKP + j2  # global pair index
                    dst_dram = AP(
                        tensor=out_ext,
                        offset=gp * ROW,
                        ap=[[NP * ROW, 128], [1, ROW]],
                    )
                    nc.sync.dma_start(dst_dram, out_sb[:])
    return nc


_CACHED = {}


def _get_graph():
    if "nc" not in _CACHED:
        _CACHED["nc"] = build_graph()
    return _CACHED["nc"]


def _pack_core(left_b, right_b, h0):
    """left_b/right_b: [C, H, W] f32 for one batch -> lrpack [128, 64, 1024] bf16.

    Layout: R row then L row contiguously (SBUF gets [pad|R|L] in one DMA);
    h-parity on partition halves (even h -> partitions 0-63, odd -> 64-127).
    """
    ls = left_b[:, h0 : h0 + HC, :]
    rs = right_b[:, h0 : h0 + HC, :]
    pack = np.empty((128, NP, 2 * W), dtype=np.float32)
    pack[0:64, :, 0:W] = rs[:, 0::2, :]
    pack[64:128, :, 0:W] = rs[:, 1::2, :]
    pack[0:64, :, W : 2 * W] = ls[:, 0::2, :]
    pack[64:128, :, W : 2 * W] = ls[:, 1::2, :]
    return pack.astype(ml_dtypes.bfloat16)


def _unpack_core(oc):
    """oc: [128, NP*ROW] bf16 -> [D, HC, W] f32 for one core.

    oc[i, ((hp*2+par)*4+t)*159 + c] = G value for h = 2hp+par,
    w = 128t + i, c = (i mod 64) + d' with d' = 95 - d.
    """
    raw = np.ascontiguousarray(oc).reshape(128, NP, 2, NT, NB)
    s_i, s_hp, s_par, s_t, s_c = raw.strides
    # axes: (hp, par, t, half, i64, d') ; i = 64*half + i64, c = i64 + d'
    band = np.lib.stride_tricks.as_strided(
        raw,
        shape=(NP, 2, NT, 2, 64, D),
        strides=(s_hp, s_par, s_t, 64 * s_i, s_i + s_c, s_c),
    )
    band = band.astype(np.float32)  # one big strided copy + upcast
    # -> [d', hp, par, t, half, i64] -> flip d' -> [D, HC, W]
    vol = band.transpose(5, 0, 1, 2, 3, 4)[::-1]
    vol = vol.reshape(D, HC, W)
    # w < d is the zero-pad region (shifted right reads past the left edge);
    # the device skips those matmul cols, so overwrite the garbage with 0
    for d in range(1, D):
        vol[d, :, :d] = 0.0
    return vol


def _run(inputs, trace=False):
    left = np.asarray(inputs["left"], dtype=np.float32)
    right = np.asarray(inputs["right"], dtype=np.float32)
    shift = np.asarray(inputs["shift"])

    nc = _get_graph()
    in_maps = []
    for core in range(8):
        b, half = core // 2, core % 2
        in_maps.append({"lrpack": _pack_core(left[b], right[b], half * HC)})

    res = run_bass_kernel_spmd(nc, in_maps, core_ids=list(range(8)), trace=trace)

    out = np.empty((B, D, H, W), dtype=np.float32)
    for core in range(8):
        b, half = core // 2, core % 2
        oc = np.asarray(res.results[core]["out"])
        out[b, :, half * HC : (half + 1) * HC, :] = _unpack_core(oc)

    # band covers integer shifts 0..95; remap if shift isn't exactly arange
    s = np.asarray(shift, dtype=np.float64)
    if not np.allclose(s, np.arange(D)):
        si = np.rint(s).astype(np.int64)
        if np.allclose(s, si) and si.min() >= 0 and si.max() < D:
            out = out[:, si, :, :]
        else:
            raise NotImplementedError(f"unsupported shift vector: {s}")
    return out, res


def kernel(**inputs) -> np.ndarray:
    out, _ = _run(inputs, trace=False)
    return out


# revision 26
# speedup vs baseline: 1.0617x; 1.0617x over previous
"""Correlation cost-volume kernel for Trainium2 (8 NeuronCores).

out[b,d,h,w] = sum_c left[b,c,h,w] * right[b,c,h,w-shift[d]]
  left/right: [4, 64, 256, 512] f32, shift: arange(96) -> out [4, 96, 256, 512] f32

Strategy:
  - Shard (b, h-half) across 8 cores: per-core left/right [64, 128, 512], no halo
    (shifts are along W only), no collectives.
  - Per (h, w-chunk of 128): two TensorEngine matmuls [K=64, M=64, N=159] bf16
    compute the Gram band G[i, c] = sum_ch L[ch, w0+i] * R[ch, r0+c] for the
    two 64-wide sub-chunks, with the upper sub-chunk's rhs window shifted by
    64 and placed at tile_position col 64, so both halves land band-ALIGNED in
    one PSUM region [128, 159] (partition i needs cols [i%64, i%64+96)).
    M=64 matters twice: the stream runs at full PE clock (M=128 streams at
    half rate - PSUM write port is 64 f32/cycle), and band alignment makes the
    extraction full-partition-width. Two h rows (even/odd) run on K-partition
    halves via tile_position rows 0/64. t=0 windows that would read w<0 are
    trimmed (no zero pad in SBUF); the host zeroes the w<d triangle instead.
  - Band extraction: ONE copy per (h-pair, parity) moves all 4 chunks
    [128, 4x159] PSUM -> SBUF bf16 (4D AP over 2 banks x 2 chunks), alternating
    Vector/Scalar engines (GpSimd cannot access PSUM).
  - Output ships as the 159-wide skewed rectangles (20.8MB/core, one DMA per
    h-pair with 2544B runs); the de-skew (banded gather) happens on the host
    with one as_strided view - no DRAM scratch roundtrip on device.
  - Host: pack/cast inputs to bf16, gather + upcast + transpose the output.
  Measured ~117us on HW (baseline 260611ns quoted / ~208754ns measured):
  PE-floor-bound (1024 matmul slots x ~114ns: 54ns stream + ~60ns LDWEIGHTS
  that this walrus build cannot hide; --enable-ldw-opt crashes its codegen),
  with DMA (37.6MB at ~360GB/s burst) just underneath.
"""
import sys

sys.path.insert(0, "/opt/trn_rl_repo")

import numpy as np
import ml_dtypes

import concourse.bass as bass
import concourse.mybir as mybir
import concourse.tile as tile
from concourse.ap import AP
from concourse.bass_utils import run_bass_kernel_spmd
from concourse.vector_clock import ScopedClock

B, C, H, W, D = 4, 64, 256, 512, 96
HC = H // 2          # 128 h rows per core
NP = HC // 2         # 64 h-pairs per core
NT = 4               # w-chunks of 128 per h row
NG = 128 + D - 1     # 223 gram cols per chunk
NB = 64 + D - 1      # 159 band-window cols per 64-row half
PAIR_COLS = W + W    # 512 R + 512 L = 1024 (no zero pad: t=0 windows are
                     # trimmed to valid cols; host zeroes the w<d triangle)
L_OFF = W            # L data starts at col 512; R at col 0
BLKP = 8             # h-pairs per input block
NBLK = NP // BLKP    # 8 blocks
ROW = 2 * NT * NB    # out_sb cols per h-pair: (par, t, c) = 2*4*159 = 1272
PS_COLS = 1024       # psum tile: 2 banks, chunks at cols 0/159/512/671

BF16 = mybir.dt.bfloat16
F32 = mybir.dt.float32


_orig_add_instruction = tile.TileContext._add_instruction


def _patched_add_instruction(self, inst):
    # This walrus build allows at most ONE sync-wait per instruction: peel
    # extra waits onto single-wait NOPs on the same engine, just before it.
    si = inst.sync_info
    if si is not None and len(si.on_wait) > 1:
        waits = list(si.on_wait)
        for w in waits[:-1]:
            nop = mybir.InstNoOp(
                name=self.nc.get_next_instruction_name(),
                text_hint="split_wait",
                bass_nofuse=True,
            )
            nop.engine = inst.engine
            nop.sync_info = mybir.SyncInfo(on_wait=[w], on_update=[])
            _orig_add_instruction(self, nop)
        si.on_wait = waits[-1:]
    _orig_add_instruction(self, inst)


tile.TileContext._add_instruction = _patched_add_instruction


def _patched_drain_and_barrier(self, tick_clock, wait_clock):
    # This walrus build allows only ONE sync-wait on the tail Drain CTRL
    # instruction; split the final-clock waits across single-wait NOPs.
    nc = self.nc
    probe = nc.sync.nop(nofuse=True, hint="drain_waits")
    wait_clock.add_sem_waits(probe.ins, ScopedClock({None: tick_clock.global_clock}))
    waits = list(probe.ins.sync_info.on_wait)
    probe.ins.sync_info.on_wait = waits[:1]
    for w in waits[1:]:
        n = nc.sync.nop(nofuse=True, hint="drain_waits")
        n.ins.sync_info = mybir.SyncInfo(on_wait=[w], on_update=[])
    nc.sync.drain()
    nc.all_engine_barrier()
    assert self.sems is not None
    popped = nc._tile_sem_poison_stack.pop()
    assert popped is self._sem_poison
    nc.clear_and_free_semaphores(list(self.sems.allocated().values()))
    nc.all_engine_barrier()


tile.TileContext._drain_and_barrier = _patched_drain_and_barrier


# chunk t lives at psum col CH_OFF[t] of a [128, PS_COLS] tile (2 banks,
# 2 chunks per bank: matmul output must not straddle a 512-f32 bank boundary)
CH_OFF = [0, NB, 512, 512 + NB]


def build_graph():
    nc = bass.Bass()
    lr_ext = nc.declare_dram_parameter("lrpack", [128, NP, 2 * W], BF16, isOutput=False)
    out_ext = nc.declare_dram_parameter("out", [128, NP * ROW], BF16, isOutput=True)

    with tile.TileContext(nc) as tc:
        with (
            tc.tile_pool(name="inp", bufs=3) as in_pool,
            tc.tile_pool(name="outsb", bufs=6) as out_pool,
            tc.tile_pool(name="psum", bufs=4, space="PSUM") as psum_pool,
        ):
            # GPSIMD/Pool cannot access PSUM (verifier); DVE + Act only
            copy_fns = [
                lambda d, s: nc.vector.tensor_copy(d, s),
                lambda d, s: nc.scalar.copy(d, s),
            ]
            ce = 0  # round-robin cursor
            for blk in range(NBLK):
                # ---- load one block: 8 h-pairs -------------------------------
                blk_tile = in_pool.tile([128, BLKP * PAIR_COLS], BF16)
                h2_0 = blk * BLKP
                # host packs R||L contiguously: one DMA, 2048-byte runs
                # (issuing on the Act HWDGE queue instead was measured slower:
                # the DMA descriptor generation contends with the band copies)
                src_rl = lr_ext[:, h2_0 : h2_0 + BLKP, :]
                nc.sync.dma_start(blk_tile[:], src_rl)

                # ---- compute: per h-pair, 2 parities x 4 w-chunks ------------
                for j2 in range(BLKP):
                    base = j2 * PAIR_COLS
                    out_sb = out_pool.tile([128, ROW], BF16)
                    for par in range(2):
                        p0 = 64 * par
                        ps = psum_pool.tile([128, PS_COLS], F32)
                        for t in range(NT):
                            w0 = 128 * t
                            for half in range(2):
                                # M=64 sub-chunk, band-aligned: rhs window is
                                # shifted by 64 for the upper half so both
                                # halves land in psum cols [CH_OFF[t], +159).
                                # t=0 windows start before R col 0: trim to the
                                # valid cols (c0 offset); the skipped psum cols
                                # are the w<d region, zeroed on the host.
                                lw = base + L_OFF + w0 + 64 * half
                                lhsT = blk_tile[p0 : p0 + 64, lw : lw + 64]
                                r0 = w0 + 64 * half - (D - 1)
                                c0 = max(0, -r0)
                                rhs = blk_tile[
                                    p0 : p0 + 64,
                                    base + r0 + c0 : base + r0 + NB,
                                ]
                                nc.tensor.matmul(
                                    ps[
                                        64 * half : 64 * half + 64,
                                        CH_OFF[t] + c0 : CH_OFF[t] + NB,
                                    ],
                                    lhsT=lhsT,
                                    rhs=rhs,
                                    start=True,
                                    stop=True,
                                    tile_position=(p0, 64 * half),
                                )
                        # one full-partition band copy per (pair, par): the 4
                        # chunks via 4D AP [(part)(bank)(chunk-in-bank)(col)]
                        pw = ps.tensor.shape[1]
                        ow = out_sb.tensor.shape[1]
                        src = AP(
                            tensor=ps.tensor,
                            offset=ps.offset,
                            ap=[[pw, 128], [512, 2], [NB, 2], [1, NB]],
                        )
                        dst = AP(
                            tensor=out_sb.tensor,
                            offset=out_sb.offset + par * NT * NB,
                            ap=[[ow, 128], [2 * NB, 2], [NB, 2], [1, NB]],
                        )
                        copy_fns[ce % 2](dst, src)
                        ce += 1
                    # one DMA per pair: contiguous 2544-B runs per partition
                    gp = blk * BLKP + j2  # global pair index
                    dst_dram = AP(
                        tensor=out_ext,
                        offset=gp * ROW,
                        ap=[[NP * ROW, 128], [1, ROW]],
                    )
                    nc.sync.dma_start(dst_dram, out_sb[:])
    return nc


_CACHED = {}


def _get_graph():
    if "nc" not in _CACHED:
        _CACHED["nc"] = build_graph()
    return _CACHED["nc"]


def _pack_core(left_b, right_b, h0):
    """left_b/right_b: [C, H, W] f32 for one batch -> lrpack [128, 64, 1024] bf16.

    Layout: R row then L row contiguously (SBUF gets [pad|R|L] in one DMA);
    h-parity on partition halves (even h -> partitions 0-63, odd -> 64-127).
    """
    ls = left_b[:, h0 : h0 + HC, :]
    rs = right_b[:, h0 : h0 + HC, :]
    pack = np.empty((128, NP, 2 * W), dtype=np.float32)
    pack[0:64, :, 0:W] = rs[:, 0::2, :]
    pack[64:128, :, 0:W] = rs[:, 1::2, :]
    pack[0:64, :, W : 2 * W] = ls[:, 0::2, :]
    pack[64:128, :, W : 2 * W] = ls[:, 1::2, :]
    return pack.astype(ml_dtypes.bfloat16)


def _unpack_core(oc):
    """oc: [128, NP*ROW] bf16 -> [D, HC, W] f32 for one core.

    oc[i, ((hp*2+par)*4+t)*159 + c] = G value for h = 2hp+par,
    w = 128t + i, c = (i mod 64) + d' with d' = 95 - d.
    """
    raw = np.ascontiguousarray(oc).reshape(128, NP, 2, NT, NB)
    s_i, s_hp, s_par, s_t, s_c = raw.strides
    # axes: (hp, par, t, half, i64, d') ; i = 64*half + i64, c = i64 + d'
    band = np.lib.stride_tricks.as_strided(
        raw,
        shape=(NP, 2, NT, 2, 64, D),
        strides=(s_hp, s_par, s_t, 64 * s_i, s_i + s_c, s_c),
    )
    band = band.astype(np.float32)  # one big strided copy + upcast
    # -> [d', hp, par, t, half, i64] -> flip d' -> [D, HC, W]
    vol = band.transpose(5, 0, 1, 2, 3, 4)[::-1]
    vol = vol.reshape(D, HC, W)
    # w < d is the zero-pad region (shifted right reads past the left edge);
    # the device skips those matmul cols, so overwrite the garbage with 0
    for d in range(1, D):
        vol[d, :, :d] = 0.0
    return vol


def _run(inputs, trace=False):
    left = np.asarray(inputs["left"], dtype=np.float32)
    right = np.asarray(inputs["right"], dtype=np.float32)
    shift = np.asarray(inputs["shift"])

    nc = _get_graph()
    in_maps = []
    for core in range(8):
        b, half = core // 2, core % 2
        in_maps.append({"lrpack": _pack_core(left[b], right[b], half * HC)})

    res = run_bass_kernel_spmd(nc, in_maps, core_ids=list(range(8)), trace=trace)

    out = np.empty((B, D, H, W), dtype=np.float32)
    for core in range(8):
        b, half = core // 2, core % 2
        oc = np.asarray(res.results[core]["out"])
        out[b, :, half * HC : (half + 1) * HC, :] = _unpack_core(oc)

    # band covers integer shifts 0..95; remap if shift isn't exactly arange
    s = np.asarray(shift, dtype=np.float64)
    if not np.allclose(s, np.arange(D)):
        si = np.rint(s).astype(np.int64)
        if np.allclose(s, si) and si.min() >= 0 and si.max() < D:
            out = out[:, si, :, :]
        else:
            raise NotImplementedError(f"unsupported shift vector: {s}")
    return out, res


def kernel(**inputs) -> np.ndarray:
    out, _ = _run(inputs, trace=False)
    return out
